# revision 69
# baseline (speedup 1.0000x reference)
# Trainium2 Bass kernel for nn_BDHBlock (dense transformer block).
#
# Strategy (8 NeuronCores, one shared SPMD program):
#   - Token-parallel for all token-local stages: core c owns flat tokens
#     [512c, 512c+512) of x.reshape(4096, 1024). LayerNorms (gamma/beta folded
#     into the following weights host-side), the pre-masked sparse linear,
#     QKV / output projections and the FFN are computed locally with
#     replicated fp16 host-pre-transposed weights.
#   - Attention is head-parallel: three split AllToAlls (k, q, v — each
#     overlapping the next projection) reshard q/k/v from token-sharded to
#     head-sharded (2 heads x full 4096-token sequence per core). Each core
#     runs exact-causal relu attention for its 2 heads over 512-wide query
#     panels with the two heads packed on disjoint PE row groups, then a
#     fourth AllToAll reshards the context back to token-sharded.
#   - Rowsums ride along as a ones-row in the v stationary; per-panel
#     normalization happens off the PE critical path.
import numpy as np

import concourse.bass as bass
import concourse.mybir as mybir
import concourse.tile as tile
from concourse import bacc
from concourse.masks import make_identity

B, S, H, NH = 2, 2048, 1024, 16
D = H // NH            # 64
FF = 4 * H             # 4096
NC = 8                 # cores
T = B * S // NC        # 512 tokens per core
TT = T // 128          # 4 token tiles
KT = H // 128          # 8 feature tiles
NFT = FF // 128        # 32
HPC = 2                # heads per core
F32, F32R, F16 = mybir.dt.float32, mybir.dt.float32r, mybir.dt.float16
ADD, SUB, MUL, MAX = (mybir.AluOpType.add, mybir.AluOpType.subtract,
                      mybir.AluOpType.mult, mybir.AluOpType.max)
AF = mybir.ActivationFunctionType
RG = [list(range(NC))]
EPS = 1e-5

_CACHE = {}


def _build():
    nc = bacc.Bacc("TRN2", target_bir_lowering=False, debug=False,
                   num_devices=NC)

    # ---------------- I/O ----------------
    def inp(name, shape, dtype=F32):
        return nc.dram_tensor(name, list(shape), dtype, kind="ExternalInput")

    x_io = inp("x_c", (T, H))
    sfwT_io = inp("sfwT", (H, H), F16)        # (sf_w*mask).T * g1, host fp16
    wT_io = {k: inp(k, (H, H), F16) for k in ("wqT", "wkT", "wvT", "woT")}
    w1T_io = inp("w1T", (H, FF), F16)
    w2T_io = inp("w2T", (FF, H), F16)
    b_io = {k: inp(k, (H,)) for k in ("sf_b", "bv", "bo", "ff2_b")}
    tri_io = inp("tri", (128, 4, 1024), F16)     # causal masks for diag tiles (x2 heads)
    bqk_col_io = inp("bqk_col", (128, 2 * KT))   # [p, 2*kt]: bq/bk per-partition cols
    ff1b_col_io = inp("ff1b_col", (128, NFT))
    out_io = nc.dram_tensor("out_c", [T, H], F32, kind="ExternalOutput")

    # internal DRAM for collectives (HBM bounce)
    SLOT = 128 * T                               # elements per (dest) slot
    CSLOT = SLOT + HPC * 512                     # ctx + rowsum rows
    k_in = nc.dram_tensor("k_in", [NC, SLOT], F16)
    k_out = nc.dram_tensor("k_out", [NC, SLOT], F16)
    q_in = nc.dram_tensor("q_in", [NC, SLOT], F16)
    q_out = nc.dram_tensor("q_out", [NC, SLOT], F16)
    v_in = nc.dram_tensor("v_in", [NC, SLOT], F16)
    v_out = nc.dram_tensor("v_out", [NC, SLOT], F16)
    cc_in = nc.dram_tensor("cc_in", [NC, CSLOT], F16)
    cc_out = nc.dram_tensor("cc_out", [NC, CSLOT], F16)
    rs_scr = nc.dram_tensor("rs_scr", [2 * KT, 512], F32)

    from contextlib import ExitStack
    with tile.TileContext(nc) as tc, ExitStack() as es:
        # ---------------- pools ----------------
        const = es.enter_context(tc.tile_pool(name="const", bufs=1))
        persist = es.enter_context(tc.tile_pool(name="persist", bufs=1))
        wpool = es.enter_context(tc.tile_pool(name="wpool", bufs=10))  # f16 [128,512]
        sc_pool = es.enter_context(tc.tile_pool(name="scratch", bufs=2))
        small = es.enter_context(tc.tile_pool(name="small", bufs=8))
        att_pool = es.enter_context(tc.tile_pool(name="attp", bufs=8))
        abuf = es.enter_context(tc.tile_pool(name="abuf", bufs=2))
        pacc = es.enter_context(tc.tile_pool(name="pacc", bufs=1, space="PSUM"))  # 2 pair tags
        pmix = es.enter_context(tc.tile_pool(name="pmix", bufs=2, space="PSUM"))

        ident = const.tile([128, 128], F32)
        make_identity(nc, ident)
        # x chunks first on the gpsimd queue: LN1 is the preamble critical path
        x_sb = persist.tile([128, TT, H], F32)
        for tt in range(TT):
            nc.gpsimd.dma_start(out=x_sb[:, tt, :],
                                in_=x_io.ap()[bass.ts(tt, 128), :])
        tri4 = const.tile([128, 4, 1024], F16)
        bqk_col = const.tile([128, 2 * KT], F32)
        nc.gpsimd.dma_start(out=bqk_col[:], in_=bqk_col_io.ap())
        ff1b_col = const.tile([128, NFT], F32)
        nc.gpsimd.dma_start(out=ff1b_col[:], in_=ff1b_col_io.ap())
        nc.gpsimd.dma_start(out=tri4[:], in_=tri_io.ap())
        eps_col = const.tile([128, 1], F32)
        nc.vector.memset(eps_col[:], EPS)

        _round = [0]
        def acc_tiles():
            r = _round[0]; _round[0] += 1
            if r % 2 == 0:
                pairs = [pacc.tile([128, 1024], F32, tag=f"acc{t}", name=f"acc{t}")
                         for t in range(2)]
            else:
                pairs = [pmix.tile([128, 1024], F32, tag="pmix", name=f"accp{t}")
                         for t in range(2)]
            return [pairs[t // 2][:, bass.ts(t % 2, 512)] for t in range(4)]

        # fp16 activation arenas (tag-shared across phases; see ordering notes)
        ln1T_sb = persist.tile([128, KT, T], F16, tag="bigA")           # -> ctxo -> h
        ln2T_sb = persist.tile([128, KT, T], F16, tag="bigB")           # -> ctxT
        qT_sb = persist.tile([128, KT, T], F16, tag="bigC")             # -> ln3T
        kT_sb = persist.tile([128, KT, T], F16, tag="bigD")             # -> v_sb
        wo_sb = persist.tile([128, KT, H], F16)                         # prefetched wo
        bias_bc = {k: persist.tile([128, H], F16, name=f"bc_{k}")
                   for k in ("sf_b", "bv", "bo", "ff2_b")}

        def bcast_row(dst, src_dram, n):
            src = src_dram.ap().unsqueeze(0).partition_broadcast(128).squeeze(1)
            nc.gpsimd.dma_start(out=dst[:, :n], in_=src)

        for k in bias_bc:
            bcast_row(bias_bc[k], b_io[k], H)
        # prefetch wo into SBUF (used only after attention; off critical path)
        nc.gpsimd.dma_start(out=wo_sb[:], in_=wT_io["woT"].ap().rearrange(
            "(kt p) n -> p kt n", p=128))

        # ---------------- layernorm (token-major, g/b folded into weights) ----
        def layer_norm_t(dst):
            for tt in range(TT):
                xt = x_sb[:, tt, :]
                sums = small.tile([128, 1], F32, tag="s0")
                sumsq = small.tile([128, 1], F32, tag="s1")
                lt = sc_pool.tile([128, H], F32, tag="lnt")
                nc.vector.reduce_sum(sums[:], xt, axis=mybir.AxisListType.X)
                nc.scalar.activation(lt[:], xt, AF.Square, accum_out=sumsq[:])
                mu = small.tile([128, 1], F32, tag="s2")
                var = small.tile([128, 1], F32, tag="s3")
                rstd = small.tile([128, 1], F32, tag="s4")
                nc.vector.tensor_scalar_mul(mu[:], sums[:], 1.0 / H)
                nc.vector.tensor_scalar_mul(var[:], sumsq[:], 1.0 / H)
                nc.vector.tensor_tensor(rstd[:], mu[:], mu[:], MUL)
                nc.vector.tensor_tensor(var[:], var[:], rstd[:], SUB)
                nc.scalar.activation(rstd[:], var[:], AF.Sqrt, bias=eps_col[:])
                nc.vector.reciprocal(rstd[:], rstd[:])
                nm = small.tile([128, 1], F32, tag="s5")
                nc.vector.scalar_tensor_tensor(nm[:], mu[:], -1.0, rstd[:],
                                               op0=MUL, op1=MUL)
                nc.vector.tensor_scalar(lt[:, 0:512], xt[:, 0:512], mu[:], rstd[:],
                                        op0=SUB, op1=MUL)
                nc.scalar.activation(lt[:, 512:], xt[:, 512:], AF.Identity,
                                     bias=nm[:], scale=rstd[:])
                for kt in range(KT):
                    pt = pmix.tile([128, 512], F32, tag="pmix", name="pt")
                    nc.tensor.transpose(pt[:, :128], lt[:, bass.ts(kt, 128)], ident[:])
                    nc.any.tensor_copy(dst[:, kt, bass.ts(tt, 128)], pt[:, :128])

        # =====================================================================
        # Stage 1: x += LN1(x) @ (sf_w * mask).T + sf_b     (mask folded host-side)
        # =====================================================================
        layer_norm_t(ln1T_sb)
        for nch in range(2):
            ps = acc_tiles()
            for kt in range(KT):
                wt = wpool.tile([128, 512], F16, tag="wa")
                nc.sync.dma_start(out=wt[:], in_=sfwT_io.ap()[bass.ts(kt, 128), bass.ts(nch, 512)])
                for tt in range(TT):
                    nc.tensor.matmul(ps[tt][:], ln1T_sb[:, kt, bass.ts(tt, 128)],
                                     wt[:], start=(kt == 0), stop=(kt == KT - 1))
            for tt in range(TT):
                xsl = x_sb[:, tt, bass.ts(nch, 512)]
                tmp = sc_pool.tile([128, 512], F32, tag="ev")
                nc.any.tensor_add(tmp[:], ps[tt][:], bias_bc["sf_b"][:, bass.ts(nch, 512)])
                nc.gpsimd.tensor_add(xsl, xsl, tmp[:])

        # =====================================================================
        # Stage 2: LN2 + K -> A2A(k) ; Q -> A2A(q) ; V -> A2A(v)
        # =====================================================================
        layer_norm_t(ln2T_sb)

        def qk_prod(wio, dst, bcol):
            # feature-major out [n 128, t 512]; scale/bias folded host-side
            for nh in range(2):
                ps = acc_tiles()
                for kt in range(KT):
                    wt = wpool.tile([128, 512], F16, tag="wa")
                    nc.sync.dma_start(out=wt[:], in_=wT_io[wio].ap()[bass.ts(kt, 128), bass.ts(nh, 512)])
                    for n4 in range(4):
                        nc.tensor.matmul(ps[n4][:], wt[:, bass.ts(n4, 128)], ln2T_sb[:, kt, :],
                                         start=(kt == 0), stop=(kt == KT - 1))
                for n4 in range(4):
                    nt = nh * 4 + n4
                    nc.any.tensor_scalar_add(dst[:, nt, :], ps[n4][:],
                                             bqk_col[:, bcol * KT + nt: bcol * KT + nt + 1])

        qk_prod("wqT", qT_sb, 0)
        nc.sync.dma_start(out=q_in.ap().rearrange("j (p t) -> p j t", p=128),
                          in_=qT_sb[:, :, :])
        nc.gpsimd.collective_compute(
            "AllToAll", mybir.AluOpType.bypass, replica_groups=RG,
            ins=[q_in.ap().opt()], outs=[q_out.ap().opt()])

        qk_prod("wkT", kT_sb, 1)
        nc.sync.dma_start(out=k_in.ap().rearrange("j (p t) -> p j t", p=128),
                          in_=kT_sb[:, :, :])
        nc.gpsimd.collective_compute(
            "AllToAll", mybir.AluOpType.bypass, replica_groups=RG,
            ins=[k_in.ap().opt()], outs=[k_out.ap().opt()])

        # v: token-major out [t 128, n 512]
        v_sb = persist.tile([128, TT, H], F16, tag="bigD", name="v_sb")
        for nch in range(2):
            ps = acc_tiles()
            for kt in range(KT):
                wt = wpool.tile([128, 512], F16, tag="wa")
                nc.sync.dma_start(out=wt[:], in_=wT_io["wvT"].ap()[bass.ts(kt, 128), bass.ts(nch, 512)])
                for tt in range(TT):
                    nc.tensor.matmul(ps[tt][:], ln2T_sb[:, kt, bass.ts(tt, 128)],
                                     wt[:], start=(kt == 0), stop=(kt == KT - 1))
            for tt in range(TT):
                nc.any.tensor_add(v_sb[:, tt, bass.ts(nch, 512)], ps[tt][:],
                                  bias_bc["bv"][:, bass.ts(nch, 512)])
        for j in range(NC):
            nc.sync.dma_start(out=v_in.ap()[j].rearrange("(p tt f) -> p tt f", p=128, tt=TT),
                              in_=v_sb[:, :, bass.ts(j, 128)])
        nc.gpsimd.collective_compute(
            "AllToAll", mybir.AluOpType.bypass, replica_groups=RG,
            ins=[v_in.ap().opt()], outs=[v_out.ap().opt()])

        # =====================================================================
        # Attention: 2 heads (row-group packed), 512-wide q panels, exact causal
        # =====================================================================
        # 65-row staging: rows 0-63 ctx, row 64 rowsums; [65, b, h, s]
        ctxs = persist.tile([D + 1, B, HPC, S], F16, tag="bigB", name="ctxs")
        for b in range(B):
            qa = abuf.tile([128, 4, T], F16, tag="qa")
            ka = abuf.tile([128, 4, T], F16, tag="ka")
            vb = abuf.tile([128, 4, TT, HPC, D + 1], F16, tag="vb")
            nc.gpsimd.dma_start(out=qa[:],
                                in_=q_out.ap()[4 * b:4 * b + 4].rearrange("i (p t) -> p i t", p=128))
            nc.gpsimd.dma_start(out=ka[:],
                                in_=k_out.ap()[4 * b:4 * b + 4].rearrange("i (p t) -> p i t", p=128))
            nc.vector.memset(vb[:, :, :, :, D:D + 1], 1.0)
            for i in range(4):
                nc.gpsimd.dma_start(
                    out=vb[:, i, :, :, 0:D],
                    in_=v_out.ap()[4 * b + i].rearrange("(p tt h d) -> p tt h d",
                                                        p=128, tt=TT, h=HPC))

            for qp in range(4):
                nkt = 4 * qp + 4
                cx = pacc.tile([128, 1024], F32, tag=f"acc{qp % 2}", name="cx")
                atts = {}

                def cx_mms(k2):
                    for h in range(HPC):
                        nc.tensor.matmul(cx[0:D + 1, bass.ts(h, 512)],
                                         vb[:, k2 // 4, k2 % 4, h, :],
                                         atts[k2][:, bass.ts(h, 512)],
                                         start=(k2 == 0), stop=(k2 == nkt - 1))
                    atts.pop(k2)

                for kt in range(nkt):
                    sc = pmix.tile([128, 1024], F32, tag="pmix", name="sc")
                    for h in range(HPC):
                        nc.tensor.matmul(sc[:, bass.ts(h, 512)],
                                         ka[bass.ts(h, 64), kt // 4, bass.ts(kt % 4, 128)],
                                         qa[bass.ts(h, 64), qp, :],
                                         start=True, stop=True)
                    att = att_pool.tile([128, HPC * 512], F16, tag="att")
                    if kt < 4 * qp:
                        nc.scalar.activation(att[:], sc[:], AF.Relu)
                    else:
                        nc.vector.scalar_tensor_tensor(
                            att[:], sc[:], 0.0, tri4[:, kt - 4 * qp, :],
                            op0=MAX, op1=MUL)
                    atts[kt] = att
                    if kt >= 3:
                        cx_mms(kt - 3)
                for k2 in range(max(0, nkt - 3), nkt):
                    cx_mms(k2)

                # evacuate ctx + rowsum rows, then stage this dest's A2A slot
                j = 4 * b + qp
                nc.vector.tensor_copy(
                    ctxs[:, b, :, bass.ds(qp * 512, 512)],
                    cx[0:D + 1, :].rearrange("p (h t) -> p h t", h=HPC))
                for h in range(HPC):
                    nc.sync.dma_start(
                        out=cc_in.ap()[j, 0:SLOT].rearrange(
                            "(p t) -> p t", p=128)[bass.ts(h, 64), :],
                        in_=ctxs[0:D, b, h, bass.ds(qp * 512, 512)])
                    nc.sync.dma_start(
                        out=cc_in.ap()[j, SLOT + h * 512: SLOT + (h + 1) * 512
                                       ].unsqueeze(0),
                        in_=ctxs[D:D + 1, b, h, bass.ds(qp * 512, 512)])

        # =====================================================================
        # A2A #4: head-sharded ctx (+rowsums) -> token-sharded; deferred norm
        # =====================================================================
        nc.gpsimd.collective_compute(
            "AllToAll", mybir.AluOpType.bypass, replica_groups=RG,
            ins=[cc_in.ap().opt()], outs=[cc_out.ap().opt()])

        # divisors: recip rowsums [16,512] -> DRAM -> partition-broadcast tiles
        # rs16h rows = h*8 + src (two whole-block loads)
        rs16h = persist.tile([16, 512], F16, name="rs16h")
        rs16 = persist.tile([16, 512], F32, name="rs16")
        for hh in range(HPC):
            nc.sync.dma_start(
                out=rs16h[8 * hh:8 * hh + 8, :],
                in_=cc_out.ap()[:, SLOT + hh * 512: SLOT + (hh + 1) * 512])
        nc.vector.tensor_scalar_add(rs16[:], rs16h[:], 1e-9)
        nc.vector.reciprocal_approx_fast(rs16[:], rs16[:])
        nc.sync.dma_start(out=rs_scr.ap(), in_=rs16[:, :])
        ctxo_sb = persist.tile([128, KT, T], F16, tag="bigA", name="ctxo_sb")
        div_sb = persist.tile([128, KT, 512], F32, name="div_sb")
        ctxo_n = persist.tile([128, KT, T], F16, tag="bigB", name="ctxo_n")
        for kt in range(KT):
            nc.gpsimd.dma_start(
                out=ctxo_sb[:, kt, :],
                in_=cc_out.ap()[:, 0:SLOT].rearrange("j (p t) -> p j t", p=128)[:, kt, :])
            for hh in range(HPC):
                src = rs_scr.ap()[8 * hh + kt].unsqueeze(0).partition_broadcast(64).squeeze(1)
                nc.sync.dma_start(out=div_sb[bass.ts(hh, 64), kt, :], in_=src)
            nc.any.tensor_tensor(ctxo_n[:, kt, :], ctxo_sb[:, kt, :],
                                 div_sb[:, kt, :], MUL)

        for nch in range(2):
            ps = acc_tiles()
            for kt in range(KT):
                for tt in range(TT):
                    nc.tensor.matmul(ps[tt][:], ctxo_n[:, kt, bass.ts(tt, 128)],
                                     wo_sb[:, kt, bass.ts(nch, 512)],
                                     start=(kt == 0), stop=(kt == KT - 1))
            for tt in range(TT):
                xsl = x_sb[:, tt, bass.ts(nch, 512)]
                tmp = sc_pool.tile([128, 512], F32, tag="ev")
                nc.any.tensor_add(tmp[:], ps[tt][:], bias_bc["bo"][:, bass.ts(nch, 512)])
                nc.gpsimd.tensor_add(xsl, xsl, tmp[:])

        # =====================================================================
        # FFN: x += relu(LN3(x) @ w1.T + b1f) @ w2.T + b2f   (g3/b3 folded)
        # =====================================================================
        ln3T_sb = persist.tile([128, KT, T], F16, tag="bigC", name="ln3T_sb")
        layer_norm_t(ln3T_sb)
        h_sb = persist.tile([128, NFT, T], F16, tag="bigA", name="h_sb")
        for nh in range(NFT // 4):
            ps = acc_tiles()
            for kt in range(KT):
                wt = wpool.tile([128, 512], F16, tag="wa")
                nc.sync.dma_start(out=wt[:], in_=w1T_io.ap()[bass.ts(kt, 128), bass.ts(nh, 512)])
                for n4 in range(4):
                    nc.tensor.matmul(ps[n4][:], wt[:, bass.ts(n4, 128)], ln3T_sb[:, kt, :],
                                     start=(kt == 0), stop=(kt == KT - 1))
            for n4 in range(4):
                nt = nh * 4 + n4
                nc.scalar.activation(h_sb[:, nt, :], ps[n4][:], AF.Relu,
                                     bias=ff1b_col[:, nt:nt + 1])
        for nch in range(2):
            ps = acc_tiles()
            for kt in range(NFT):
                wt = wpool.tile([128, 512], F16, tag="wa")
                nc.sync.dma_start(out=wt[:], in_=w2T_io.ap()[bass.ts(kt, 128), bass.ts(nch, 512)])
                for tt in range(TT):
                    nc.tensor.matmul(ps[tt][:], h_sb[:, kt, bass.ts(tt, 128)],
                                     wt[:], start=(kt == 0), stop=(kt == NFT - 1))
            for tt in range(TT):
                xsl = x_sb[:, tt, bass.ts(nch, 512)]
                tmp = sc_pool.tile([128, 512], F32, tag="ev")
                nc.any.tensor_add(tmp[:], ps[tt][:], bias_bc["ff2_b"][:, bass.ts(nch, 512)])
                nc.gpsimd.tensor_add(xsl, xsl, tmp[:])
                nc.sync.dma_start(
                    out=out_io.ap()[bass.ts(tt, 128), bass.ts(nch, 512)],
                    in_=x_sb[:, tt, bass.ts(nch, 512)])

    nc.compile()
    return nc


def _prep_shared(inputs):
    f = lambda a: np.ascontiguousarray(np.asarray(a, np.float32))
    g1, b1 = f(inputs["g1"]), f(inputs["b1"])
    g2, b2 = f(inputs["g2"]), f(inputs["b2"])
    g3, b3 = f(inputs["g3"]), f(inputs["b3"])
    A = f(inputs["sf_w"]) * f(inputs["mask"])
    wq, wk, wv, wo = f(inputs["wq"]), f(inputs["wk"]), f(inputs["wv"]), f(inputs["wo"])
    w1, w2 = f(inputs["ff1_w"]), f(inputs["ff2_w"])
    qsc = 1.0 / float(np.sqrt(np.sqrt(D)))
    c16 = lambda a: np.ascontiguousarray(a.astype(np.float16))
    sh = {
        "sfwT": c16(A.T * g1[:, None]),
        "wqT": c16(wq.T * g2[:, None] * qsc),
        "wkT": c16(wk.T * g2[:, None] * qsc),
        "wvT": c16(wv.T * g2[:, None]),
        "woT": c16(wo.T),
        "w1T": c16(w1.T * g3[:, None]),
        "w2T": c16(w2.T),
        "sf_b": f(inputs["sf_b"]) + A @ b1,
        "bv": f(inputs["bv"]) + wv @ b2,
        "bo": f(inputs["bo"]),
        "ff2_b": f(inputs["ff2_b"]),
    }
    bq_eff = (f(inputs["bq"]) + wq @ b2) * qsc
    bk_eff = (f(inputs["bk"]) + wk @ b2) * qsc
    ff1b_eff = f(inputs["ff1_b"]) + w1 @ b3
    sh["bqk_col"] = np.ascontiguousarray(
        np.stack([bq_eff, bk_eff]).reshape(2 * KT, 128).T)
    sh["ff1b_col"] = np.ascontiguousarray(ff1b_eff.reshape(NFT, 128).T)
    t4 = np.zeros((128, 4, 1024), np.float16)
    for i in range(4):
        m = (np.arange(128)[:, None] + 128 * i) <= np.arange(512)[None, :]
        t4[:, i, 0:512] = m
        t4[:, i, 512:] = m
    sh["tri"] = t4
    return sh


def kernel(**inputs) -> np.ndarray:
    from concourse.bass_utils import run_bass_kernel_spmd

    if "nc" not in _CACHE:
        _CACHE["nc"] = _build()
    nc = _CACHE["nc"]

    sh = _prep_shared(inputs)
    x = np.ascontiguousarray(np.asarray(inputs["x"], np.float32)).reshape(B * S, H)
    in_maps = []
    for c in range(NC):
        m = dict(sh)
        m["x_c"] = np.ascontiguousarray(x[c * T:(c + 1) * T])
        in_maps.append(m)

    res = run_bass_kernel_spmd(nc, in_maps, core_ids=list(range(NC)))
    out = np.concatenate([res.results[c]["out_c"] for c in range(NC)], axis=0)
    return out.reshape(B, S, H).astype(np.float32)


# revision 71
# speedup vs baseline: 1.0905x; 1.0905x over previous
# Trainium2 Bass kernel for nn_BDHBlock (dense transformer block).
#
# Strategy (8 NeuronCores, one shared SPMD program):
#   - Token-parallel for all token-local stages: core c owns flat tokens
#     [512c, 512c+512) of x.reshape(4096, 1024). LayerNorms (gamma/beta folded
#     into the following weights host-side), the pre-masked sparse linear,
#     QKV / output projections and the FFN are computed locally with
#     replicated fp16 host-pre-transposed weights.
#   - Attention is head-parallel: three split AllToAlls (k, q, v — each
#     overlapping the next projection) reshard q/k/v from token-sharded to
#     head-sharded (2 heads x full 4096-token sequence per core). Each core
#     runs exact-causal relu attention for its 2 heads over 512-wide query
#     panels with the two heads packed on disjoint PE row groups, then a
#     fourth AllToAll reshards the context back to token-sharded.
#   - Rowsums ride along as a ones-row in the v stationary; per-panel
#     normalization happens off the PE critical path.
import numpy as np

import concourse.bass as bass
import concourse.mybir as mybir
import concourse.tile as tile
from concourse import bacc
from concourse.masks import make_identity

B, S, H, NH = 2, 2048, 1024, 16
D = H // NH            # 64
FF = 4 * H             # 4096
NC = 8                 # cores
T = B * S // NC        # 512 tokens per core
TT = T // 128          # 4 token tiles
KT = H // 128          # 8 feature tiles
NFT = FF // 128        # 32
HPC = 2                # heads per core
F32, F32R, F16 = mybir.dt.float32, mybir.dt.float32r, mybir.dt.float16
ADD, SUB, MUL, MAX = (mybir.AluOpType.add, mybir.AluOpType.subtract,
                      mybir.AluOpType.mult, mybir.AluOpType.max)
AF = mybir.ActivationFunctionType
RG = [list(range(NC))]
EPS = 1e-5

_CACHE = {}


def _build():
    nc = bacc.Bacc("TRN2", target_bir_lowering=False, debug=False,
                   num_devices=NC)

    # ---------------- I/O ----------------
    def inp(name, shape, dtype=F32):
        return nc.dram_tensor(name, list(shape), dtype, kind="ExternalInput")

    x_io = inp("x_c", (T, H))
    sfwT_io = inp("sfwT", (H, H), F16)        # (sf_w*mask).T * g1, host fp16
    wT_io = {k: inp(k, (H, H), F16) for k in ("wqT", "wkT", "wvT", "woT")}
    w1T_io = inp("w1T", (H, FF), F16)
    w2T_io = inp("w2T", (FF, H), F16)
    b_io = {k: inp(k, (H,)) for k in ("sf_b", "bv", "bo", "ff2_b")}
    tri_io = inp("tri", (128, 4, 1024), F16)     # causal masks for diag tiles (x2 heads)
    bqk_col_io = inp("bqk_col", (128, 2 * KT))   # [p, 2*kt]: bq/bk per-partition cols
    ff1b_col_io = inp("ff1b_col", (128, NFT))
    out_io = nc.dram_tensor("out_c", [T, H], F32, kind="ExternalOutput")

    # internal DRAM for collectives (HBM bounce)
    SLOT = 128 * T                               # elements per (dest) slot
    CSLOT = SLOT + HPC * 512                     # ctx + rowsum rows
    k_in = nc.dram_tensor("k_in", [NC, SLOT], F16)
    k_out = nc.dram_tensor("k_out", [NC, SLOT], F16)
    q_in = nc.dram_tensor("q_in", [NC, SLOT], F16)
    q_out = nc.dram_tensor("q_out", [NC, SLOT], F16)
    v_in = nc.dram_tensor("v_in", [NC, SLOT], F16)
    v_out = nc.dram_tensor("v_out", [NC, SLOT], F16)
    cc_in = nc.dram_tensor("cc_in", [NC, CSLOT], F16)
    cc_out = nc.dram_tensor("cc_out", [NC, CSLOT], F16)
    rs_scr = nc.dram_tensor("rs_scr", [2 * KT, 512], F32)

    from contextlib import ExitStack
    with tile.TileContext(nc) as tc, ExitStack() as es:
        # ---------------- pools ----------------
        const = es.enter_context(tc.tile_pool(name="const", bufs=1))
        persist = es.enter_context(tc.tile_pool(name="persist", bufs=1))
        wpool = es.enter_context(tc.tile_pool(name="wpool", bufs=10))  # f16 [128,512]
        sc_pool = es.enter_context(tc.tile_pool(name="scratch", bufs=2))
        small = es.enter_context(tc.tile_pool(name="small", bufs=8))
        att_pool = es.enter_context(tc.tile_pool(name="attp", bufs=8))
        abuf = es.enter_context(tc.tile_pool(name="abuf", bufs=2))
        pacc = es.enter_context(tc.tile_pool(name="pacc", bufs=1, space="PSUM"))  # 2 pair tags
        pmix = es.enter_context(tc.tile_pool(name="pmix", bufs=2, space="PSUM"))

        ident = const.tile([128, 128], F32)
        make_identity(nc, ident)
        tri4 = const.tile([128, 4, 1024], F16)
        nc.gpsimd.dma_start(out=tri4[:], in_=tri_io.ap())
        bqk_col = const.tile([128, 2 * KT], F32)
        nc.gpsimd.dma_start(out=bqk_col[:], in_=bqk_col_io.ap())
        ff1b_col = const.tile([128, NFT], F32)
        nc.gpsimd.dma_start(out=ff1b_col[:], in_=ff1b_col_io.ap())
        eps_col = const.tile([128, 1], F32)
        nc.vector.memset(eps_col[:], EPS)

        _round = [0]
        def acc_tiles():
            r = _round[0]; _round[0] += 1
            if r % 2 == 0:
                pairs = [pacc.tile([128, 1024], F32, tag=f"acc{t}", name=f"acc{t}")
                         for t in range(2)]
            else:
                pairs = [pmix.tile([128, 1024], F32, tag="pmix", name=f"accp{t}")
                         for t in range(2)]
            return [pairs[t // 2][:, bass.ts(t % 2, 512)] for t in range(4)]

        # residual stream, token-major [128, tt, H]; per-tt loads
        x_sb = persist.tile([128, TT, H], F32)
        for tt in range(TT):
            nc.sync.dma_start(out=x_sb[:, tt, :],
                              in_=x_io.ap()[bass.ts(tt, 128), :])

        # fp16 activation arenas (tag-shared across phases; see ordering notes)
        ln1T_sb = persist.tile([128, KT, T], F16, tag="bigA")           # -> ctxo -> h
        ln2T_sb = persist.tile([128, KT, T], F16, tag="bigB")           # -> ctxT
        qT_sb = persist.tile([128, KT, T], F16, tag="bigC")             # -> ln3T
        kT_sb = persist.tile([128, KT, T], F16, tag="bigD")             # -> v_sb
        wo_sb = persist.tile([128, KT, H], F16)                         # prefetched wo
        bias_bc = {k: persist.tile([128, H], F16, name=f"bc_{k}")
                   for k in ("sf_b", "bv", "bo", "ff2_b")}

        def bcast_row(dst, src_dram, n):
            src = src_dram.ap().unsqueeze(0).partition_broadcast(128).squeeze(1)
            nc.gpsimd.dma_start(out=dst[:, :n], in_=src)

        for k in bias_bc:
            bcast_row(bias_bc[k], b_io[k], H)
        # prefetch wo into SBUF (used only after attention; off critical path)
        nc.gpsimd.dma_start(out=wo_sb[:], in_=wT_io["woT"].ap().rearrange(
            "(kt p) n -> p kt n", p=128))

        # ---------------- layernorm (token-major, g/b folded into weights) ----
        def layer_norm_t(dst):
            for tt in range(TT):
                xt = x_sb[:, tt, :]
                sums = small.tile([128, 1], F32, tag="s0")
                sumsq = small.tile([128, 1], F32, tag="s1")
                lt = sc_pool.tile([128, H], F32, tag="lnt")
                nc.vector.reduce_sum(sums[:], xt, axis=mybir.AxisListType.X)
                nc.scalar.activation(lt[:], xt, AF.Square, accum_out=sumsq[:])
                mu = small.tile([128, 1], F32, tag="s2")
                var = small.tile([128, 1], F32, tag="s3")
                rstd = small.tile([128, 1], F32, tag="s4")
                nc.vector.tensor_scalar_mul(mu[:], sums[:], 1.0 / H)
                nc.vector.tensor_scalar_mul(var[:], sumsq[:], 1.0 / H)
                nc.vector.tensor_tensor(rstd[:], mu[:], mu[:], MUL)
                nc.vector.tensor_tensor(var[:], var[:], rstd[:], SUB)
                nc.scalar.activation(rstd[:], var[:], AF.Sqrt, bias=eps_col[:])
                nc.vector.reciprocal(rstd[:], rstd[:])
                nm = small.tile([128, 1], F32, tag="s5")
                nc.vector.scalar_tensor_tensor(nm[:], mu[:], -1.0, rstd[:],
                                               op0=MUL, op1=MUL)
                nc.vector.tensor_scalar(lt[:, 0:512], xt[:, 0:512], mu[:], rstd[:],
                                        op0=SUB, op1=MUL)
                nc.scalar.activation(lt[:, 512:], xt[:, 512:], AF.Identity,
                                     bias=nm[:], scale=rstd[:])
                for kt in range(KT):
                    pt = pmix.tile([128, 512], F32, tag="pmix", name="pt")
                    nc.tensor.transpose(pt[:, :128], lt[:, bass.ts(kt, 128)], ident[:])
                    nc.any.tensor_copy(dst[:, kt, bass.ts(tt, 128)], pt[:, :128])

        # =====================================================================
        # Stage 1: x += LN1(x) @ (sf_w * mask).T + sf_b     (mask folded host-side)
        # =====================================================================
        layer_norm_t(ln1T_sb)
        for nch in range(2):
            ps = acc_tiles()
            for kt in range(KT):
                wt = wpool.tile([128, 512], F16, tag="wa")
                nc.sync.dma_start(out=wt[:], in_=sfwT_io.ap()[bass.ts(kt, 128), bass.ts(nch, 512)])
                for tt in range(TT):
                    nc.tensor.matmul(ps[tt][:], ln1T_sb[:, kt, bass.ts(tt, 128)],
                                     wt[:], start=(kt == 0), stop=(kt == KT - 1))
            for tt in range(TT):
                xsl = x_sb[:, tt, bass.ts(nch, 512)]
                tmp = sc_pool.tile([128, 512], F32, tag="ev")
                nc.any.tensor_add(tmp[:], ps[tt][:], bias_bc["sf_b"][:, bass.ts(nch, 512)])
                nc.gpsimd.tensor_add(xsl, xsl, tmp[:])

        # =====================================================================
        # Stage 2: LN2 + K -> A2A(k) ; Q -> A2A(q) ; V -> A2A(v)
        # =====================================================================
        layer_norm_t(ln2T_sb)

        def qk_prod(wio, dst, bcol):
            # feature-major out [n 128, t 512]; scale/bias folded host-side
            for nh in range(2):
                ps = acc_tiles()
                for kt in range(KT):
                    wt = wpool.tile([128, 512], F16, tag="wa")
                    nc.sync.dma_start(out=wt[:], in_=wT_io[wio].ap()[bass.ts(kt, 128), bass.ts(nh, 512)])
                    for n4 in range(4):
                        nc.tensor.matmul(ps[n4][:], wt[:, bass.ts(n4, 128)], ln2T_sb[:, kt, :],
                                         start=(kt == 0), stop=(kt == KT - 1))
                for n4 in range(4):
                    nt = nh * 4 + n4
                    nc.any.tensor_scalar_add(dst[:, nt, :], ps[n4][:],
                                             bqk_col[:, bcol * KT + nt: bcol * KT + nt + 1])

        qk_prod("wqT", qT_sb, 0)
        nc.sync.dma_start(out=q_in.ap().rearrange("j (p t) -> p j t", p=128),
                          in_=qT_sb[:, :, :])
        nc.gpsimd.collective_compute(
            "AllToAll", mybir.AluOpType.bypass, replica_groups=RG,
            ins=[q_in.ap().opt()], outs=[q_out.ap().opt()])

        qk_prod("wkT", kT_sb, 1)
        nc.sync.dma_start(out=k_in.ap().rearrange("j (p t) -> p j t", p=128),
                          in_=kT_sb[:, :, :])
        nc.gpsimd.collective_compute(
            "AllToAll", mybir.AluOpType.bypass, replica_groups=RG,
            ins=[k_in.ap().opt()], outs=[k_out.ap().opt()])

        # v: token-major out [t 128, n 512]
        v_sb = persist.tile([128, TT, H], F16, tag="bigD", name="v_sb")
        for nch in range(2):
            ps = acc_tiles()
            for kt in range(KT):
                wt = wpool.tile([128, 512], F16, tag="wa")
                nc.sync.dma_start(out=wt[:], in_=wT_io["wvT"].ap()[bass.ts(kt, 128), bass.ts(nch, 512)])
                for tt in range(TT):
                    nc.tensor.matmul(ps[tt][:], ln2T_sb[:, kt, bass.ts(tt, 128)],
                                     wt[:], start=(kt == 0), stop=(kt == KT - 1))
            for tt in range(TT):
                nc.any.tensor_add(v_sb[:, tt, bass.ts(nch, 512)], ps[tt][:],
                                  bias_bc["bv"][:, bass.ts(nch, 512)])
        for j in range(NC):
            nc.sync.dma_start(out=v_in.ap()[j].rearrange("(p tt f) -> p tt f", p=128, tt=TT),
                              in_=v_sb[:, :, bass.ts(j, 128)])
        nc.gpsimd.collective_compute(
            "AllToAll", mybir.AluOpType.bypass, replica_groups=RG,
            ins=[v_in.ap().opt()], outs=[v_out.ap().opt()])

        # =====================================================================
        # Attention: 2 heads (row-group packed), 512-wide q panels, exact causal
        # =====================================================================
        # 65-row staging: rows 0-63 ctx, row 64 rowsums; [65, b, h, s]
        ctxs = persist.tile([D + 1, B, HPC, S], F16, tag="bigB", name="ctxs")
        for b in range(B):
            qa = abuf.tile([128, 4, T], F16, tag="qa")
            ka = abuf.tile([128, 4, T], F16, tag="ka")
            vb = abuf.tile([128, 4, TT, HPC, D + 1], F16, tag="vb")
            nc.gpsimd.dma_start(out=qa[:],
                                in_=q_out.ap()[4 * b:4 * b + 4].rearrange("i (p t) -> p i t", p=128))
            nc.gpsimd.dma_start(out=ka[:],
                                in_=k_out.ap()[4 * b:4 * b + 4].rearrange("i (p t) -> p i t", p=128))
            nc.vector.memset(vb[:, :, :, :, D:D + 1], 1.0)
            for i in range(4):
                nc.gpsimd.dma_start(
                    out=vb[:, i, :, :, 0:D],
                    in_=v_out.ap()[4 * b + i].rearrange("(p tt h d) -> p tt h d",
                                                        p=128, tt=TT, h=HPC))

            for qp in range(4):
                nkt = 4 * qp + 4
                cx = pacc.tile([128, 1024], F32, tag=f"acc{qp % 2}", name="cx")
                atts = {}

                def cx_mms(k2):
                    for h in range(HPC):
                        nc.tensor.matmul(cx[0:D + 1, bass.ts(h, 512)],
                                         vb[:, k2 // 4, k2 % 4, h, :],
                                         atts[k2][:, bass.ts(h, 512)],
                                         start=(k2 == 0), stop=(k2 == nkt - 1))
                    atts.pop(k2)

                for kt in range(nkt):
                    sc = pmix.tile([128, 1024], F32, tag="pmix", name="sc")
                    for h in range(HPC):
                        nc.tensor.matmul(sc[:, bass.ts(h, 512)],
                                         ka[bass.ts(h, 64), kt // 4, bass.ts(kt % 4, 128)],
                                         qa[bass.ts(h, 64), qp, :],
                                         start=True, stop=True)
                    att = att_pool.tile([128, HPC * 512], F16, tag="att")
                    if kt < 4 * qp:
                        nc.scalar.activation(att[:], sc[:], AF.Relu)
                    else:
                        nc.vector.scalar_tensor_tensor(
                            att[:], sc[:], 0.0, tri4[:, kt - 4 * qp, :],
                            op0=MAX, op1=MUL)
                    atts[kt] = att
                    if kt >= 3:
                        cx_mms(kt - 3)
                for k2 in range(max(0, nkt - 3), nkt):
                    cx_mms(k2)

                # evacuate ctx + rowsum rows, then stage this dest's A2A slot
                j = 4 * b + qp
                nc.vector.tensor_copy(
                    ctxs[:, b, :, bass.ds(qp * 512, 512)],
                    cx[0:D + 1, :].rearrange("p (h t) -> p h t", h=HPC))
                for h in range(HPC):
                    nc.sync.dma_start(
                        out=cc_in.ap()[j, 0:SLOT].rearrange(
                            "(p t) -> p t", p=128)[bass.ts(h, 64), :],
                        in_=ctxs[0:D, b, h, bass.ds(qp * 512, 512)])
                    nc.sync.dma_start(
                        out=cc_in.ap()[j, SLOT + h * 512: SLOT + (h + 1) * 512
                                       ].unsqueeze(0),
                        in_=ctxs[D:D + 1, b, h, bass.ds(qp * 512, 512)])

        # =====================================================================
        # A2A #4: head-sharded ctx (+rowsums) -> token-sharded; deferred norm
        # =====================================================================
        nc.gpsimd.collective_compute(
            "AllToAll", mybir.AluOpType.bypass, replica_groups=RG,
            ins=[cc_in.ap().opt()], outs=[cc_out.ap().opt()])

        # divisors: recip rowsums [16,512] -> DRAM -> partition-broadcast tiles
        # rs16h rows = h*8 + src (two whole-block loads)
        rs16h = persist.tile([16, 512], F16, name="rs16h")
        rs16 = persist.tile([16, 512], F32, name="rs16")
        for hh in range(HPC):
            nc.sync.dma_start(
                out=rs16h[8 * hh:8 * hh + 8, :],
                in_=cc_out.ap()[:, SLOT + hh * 512: SLOT + (hh + 1) * 512])
        nc.vector.tensor_scalar_add(rs16[:], rs16h[:], 1e-9)
        nc.vector.reciprocal_approx_fast(rs16[:], rs16[:])
        nc.sync.dma_start(out=rs_scr.ap(), in_=rs16[:, :])
        ctxo_sb = persist.tile([128, KT, T], F16, tag="bigA", name="ctxo_sb")
        div_sb = persist.tile([128, KT, 512], F32, name="div_sb")
        ctxo_n = persist.tile([128, KT, T], F16, tag="bigB", name="ctxo_n")
        for kt in range(KT):
            nc.gpsimd.dma_start(
                out=ctxo_sb[:, kt, :],
                in_=cc_out.ap()[:, 0:SLOT].rearrange("j (p t) -> p j t", p=128)[:, kt, :])
            for hh in range(HPC):
                src = rs_scr.ap()[8 * hh + kt].unsqueeze(0).partition_broadcast(64).squeeze(1)
                nc.sync.dma_start(out=div_sb[bass.ts(hh, 64), kt, :], in_=src)
            nc.any.tensor_tensor(ctxo_n[:, kt, :], ctxo_sb[:, kt, :],
                                 div_sb[:, kt, :], MUL)

        for nch in range(2):
            ps = acc_tiles()
            for kt in range(KT):
                for tt in range(TT):
                    nc.tensor.matmul(ps[tt][:], ctxo_n[:, kt, bass.ts(tt, 128)],
                                     wo_sb[:, kt, bass.ts(nch, 512)],
                                     start=(kt == 0), stop=(kt == KT - 1))
            for tt in range(TT):
                xsl = x_sb[:, tt, bass.ts(nch, 512)]
                tmp = sc_pool.tile([128, 512], F32, tag="ev")
                nc.any.tensor_add(tmp[:], ps[tt][:], bias_bc["bo"][:, bass.ts(nch, 512)])
                nc.gpsimd.tensor_add(xsl, xsl, tmp[:])

        # =====================================================================
        # FFN: x += relu(LN3(x) @ w1.T + b1f) @ w2.T + b2f   (g3/b3 folded)
        # =====================================================================
        ln3T_sb = persist.tile([128, KT, T], F16, tag="bigC", name="ln3T_sb")
        layer_norm_t(ln3T_sb)
        h_sb = persist.tile([128, NFT, T], F16, tag="bigA", name="h_sb")
        for nh in range(NFT // 4):
            ps = acc_tiles()
            for kt in range(KT):
                wt = wpool.tile([128, 512], F16, tag="wa")
                nc.sync.dma_start(out=wt[:], in_=w1T_io.ap()[bass.ts(kt, 128), bass.ts(nh, 512)])
                for n4 in range(4):
                    nc.tensor.matmul(ps[n4][:], wt[:, bass.ts(n4, 128)], ln3T_sb[:, kt, :],
                                     start=(kt == 0), stop=(kt == KT - 1))
            for n4 in range(4):
                nt = nh * 4 + n4
                nc.scalar.activation(h_sb[:, nt, :], ps[n4][:], AF.Relu,
                                     bias=ff1b_col[:, nt:nt + 1])
        for nch in range(2):
            ps = acc_tiles()
            for kt in range(NFT):
                wt = wpool.tile([128, 512], F16, tag="wa")
                nc.sync.dma_start(out=wt[:], in_=w2T_io.ap()[bass.ts(kt, 128), bass.ts(nch, 512)])
                for tt in range(TT):
                    nc.tensor.matmul(ps[tt][:], h_sb[:, kt, bass.ts(tt, 128)],
                                     wt[:], start=(kt == 0), stop=(kt == NFT - 1))
            for tt in range(TT):
                xsl = x_sb[:, tt, bass.ts(nch, 512)]
                tmp = sc_pool.tile([128, 512], F32, tag="ev")
                nc.any.tensor_add(tmp[:], ps[tt][:], bias_bc["ff2_b"][:, bass.ts(nch, 512)])
                nc.gpsimd.tensor_add(xsl, xsl, tmp[:])
                nc.sync.dma_start(
                    out=out_io.ap()[bass.ts(tt, 128), bass.ts(nch, 512)],
                    in_=x_sb[:, tt, bass.ts(nch, 512)])

    nc.compile()
    return nc


def _prep_shared(inputs):
    f = lambda a: np.ascontiguousarray(np.asarray(a, np.float32))
    g1, b1 = f(inputs["g1"]), f(inputs["b1"])
    g2, b2 = f(inputs["g2"]), f(inputs["b2"])
    g3, b3 = f(inputs["g3"]), f(inputs["b3"])
    A = f(inputs["sf_w"]) * f(inputs["mask"])
    wq, wk, wv, wo = f(inputs["wq"]), f(inputs["wk"]), f(inputs["wv"]), f(inputs["wo"])
    w1, w2 = f(inputs["ff1_w"]), f(inputs["ff2_w"])
    qsc = 1.0 / float(np.sqrt(np.sqrt(D)))
    c16 = lambda a: np.ascontiguousarray(a.astype(np.float16))
    sh = {
        "sfwT": c16(A.T * g1[:, None]),
        "wqT": c16(wq.T * g2[:, None] * qsc),
        "wkT": c16(wk.T * g2[:, None] * qsc),
        "wvT": c16(wv.T * g2[:, None]),
        "woT": c16(wo.T),
        "w1T": c16(w1.T * g3[:, None]),
        "w2T": c16(w2.T),
        "sf_b": f(inputs["sf_b"]) + A @ b1,
        "bv": f(inputs["bv"]) + wv @ b2,
        "bo": f(inputs["bo"]),
        "ff2_b": f(inputs["ff2_b"]),
    }
    bq_eff = (f(inputs["bq"]) + wq @ b2) * qsc
    bk_eff = (f(inputs["bk"]) + wk @ b2) * qsc
    ff1b_eff = f(inputs["ff1_b"]) + w1 @ b3
    sh["bqk_col"] = np.ascontiguousarray(
        np.stack([bq_eff, bk_eff]).reshape(2 * KT, 128).T)
    sh["ff1b_col"] = np.ascontiguousarray(ff1b_eff.reshape(NFT, 128).T)
    t4 = np.zeros((128, 4, 1024), np.float16)
    for i in range(4):
        m = (np.arange(128)[:, None] + 128 * i) <= np.arange(512)[None, :]
        t4[:, i, 0:512] = m
        t4[:, i, 512:] = m
    sh["tri"] = t4
    return sh


def kernel(**inputs) -> np.ndarray:
    from concourse.bass_utils import run_bass_kernel_spmd

    if "nc" not in _CACHE:
        _CACHE["nc"] = _build()
    nc = _CACHE["nc"]

    sh = _prep_shared(inputs)
    x = np.ascontiguousarray(np.asarray(inputs["x"], np.float32)).reshape(B * S, H)
    in_maps = []
    for c in range(NC):
        m = dict(sh)
        m["x_c"] = np.ascontiguousarray(x[c * T:(c + 1) * T])
        in_maps.append(m)

    res = run_bass_kernel_spmd(nc, in_maps, core_ids=list(range(NC)))
    out = np.concatenate([res.results[c]["out_c"] for c in range(NC)], axis=0)
    return out.reshape(B, S, H).astype(np.float32)
